# revision 15
# baseline (speedup 1.0000x reference)
"""Trainium2 Bass kernel for nn_AttentionCIDNN (block-diagonal crowd attention).

Problem: x[8192, 8, 2] -> last timestep -> 3-layer MLP -> h[8192, 64];
128 groups of 64 agents; per group A = h_g @ h_g^T, column-shifted softmax
P = exp(A - m[j]) / (sum_j exp(A - m[j]) + eps); scatter P onto the block
diagonal of an 8192 x 8192 zero matrix.

Sharding: 8 cores, each owns 1024 contiguous agents (16 groups). The output
is block-diagonal: only the 16 nonzero 64x64 blocks per core are computed.

Key algebra: A is bitwise symmetric on the PE (same contraction order for
[i,j] and [j,i]), and the reference's m[j] is the row-max, so
E = exp(A - m[j]) = G^T where G = exp(A - rowmax[i]) -- a PER-PARTITION
shift (cheap 0-stride broadcast) instead of a cross-partition one. The
device ships G; the host pastes each 64x64 block transposed and applies the
row normalization E/(sum+eps) during assembly.

Structure per core:
- two input DMAs on the gpsimd SWDGE queue, triggered raw right after the
  engine preamble; a dummy activation preloads the scalar ACT table during
  the DMA wait.
- all biases are folded into the matmuls via ones-row augmentation (host
  packs [x_hi;x_lo;x_hi;x_lo;1;1] against [W1_hi;W1_hi;W1_lo;W1_lo;b1_hi;
  b1_lo] for an exact bf16 L1; W2/W3 get ones-rows in h via a DMA'd const
  row), so the relus are bias-free and balance across vector/scalar.
- L2/L3/attention matmuls are true fp32: exp() amplifies any error in A
  (|A| up to ~168); bf16 or float32r anywhere in that chain pushes max rel
  err past the 2e-2 gate (measured 2.2e-2 with fp32r L2/L3).
- MLP in four 256-col chunks; each chunk's activations overlap the next
  chunk's matmuls; each chunk's 4 attention blocks and its softmax quarter
  (rowmax -> subtract -> exp -> DMA out) follow immediately and overlap
  later chunks' PE work.

Self-contained: hardcodes all shapes; builds the Bass graph once per process.
"""

import os
os.environ.setdefault("JAX_PLATFORMS", "axon")  # device exec path under axon

import numpy as np

import concourse.bass as bass
import concourse.bacc as bacc
import concourse.mybir as mybir
from concourse.tile import TileContext
from concourse.bass_utils import run_bass_kernel_spmd

F32 = mybir.dt.float32
F32R = mybir.dt.float32r
BF16 = mybir.dt.bfloat16

BS = 8192          # total agents
NCORES = 8
AGENTS = BS // NCORES   # 1024 agents per core
BLK = 64                # agents per attention group
EPS = np.float32(1e-7)
NCH = 4
CHUNK = AGENTS // NCH   # 256: MLP chunk = softmax quarter = 4 blocks

# xws (bf16): [10, 1056] = K-stacked exact-f32 split of [xT; 1] against
#   [W1; b1]: rows 0:2 x_hi, 2:4 x_lo, 4:6 x_hi, 6:8 x_lo, 8 ones, 9 ones
#   paired with w1s rows W1_hi, W1_hi, W1_lo, W1_lo, b1_hi, b1_lo.
XWS_COLS = AGENTS + 32
# wb (f32): [65, 128] = W3a [65, 0:64] | W2a rows 0:33 [64:128]
#   (Wka = [Wk; bk^T], consumed against h with a trailing ones-row)
WB_COLS = 128

_NC_CACHE = None
LAST_RESULT = None  # BassKernelResults of the most recent run (for test harness)


def build_nc():
    """Build the single-core Bass graph (identical on all 8 cores)."""
    nc = bacc.Bacc("TRN2", target_bir_lowering=False)

    xws = nc.declare_dram_parameter("xws", [10, XWS_COLS], BF16,
                                    isOutput=False)
    wb = nc.declare_dram_parameter("wb", [65, WB_COLS], F32, isOutput=False)
    ones = nc.declare_dram_parameter("ones", [1, AGENTS], F32, isOutput=False)
    bands = nc.declare_dram_parameter("bands", [64, 16 * BLK], F32,
                                      isOutput=True)

    # ---- input DMAs on the gpsimd SWDGE queue, emitted raw so they trigger
    # right after the engine preamble instead of behind the tile-pool entry.
    isem = nc.alloc_semaphore("inp")
    xws_s = nc.alloc_sbuf_tensor("xws_s", [10, XWS_COLS], BF16)
    wb_s = nc.alloc_sbuf_tensor("wb_s", [65, WB_COLS], F32)
    nc.gpsimd.dma_start(out=xws_s[:, :], in_=xws[:, :]).then_inc(isem, 16)
    nc.gpsimd.dma_start(out=wb_s[:, :], in_=wb[:, :]).then_inc(isem, 16)

    w1s_s = xws_s[:, AGENTS:AGENTS + 32]
    w3a_s = wb_s[0:65, 0:64]
    w2a_s = wb_s[0:33, 64:128]

    # scalar: preload the ACT table (1.3us) while the input DMAs fly; the
    # scratch tile is uninitialized, the result is never read.
    scr = nc.alloc_sbuf_tensor("scr", [1, 8], F32)
    scr2 = nc.alloc_sbuf_tensor("scr2", [1, 8], F32)
    nc.scalar.activation(scr2[:, :], scr[:, :],
                         mybir.ActivationFunctionType.Relu)

    # PE warm-up: the PE runs at 1.2 GHz until its free-running activity
    # window sees ~3.4us of continuous matmul traffic, then doubles to
    # 2.4 GHz. Burn the input-DMA wait on dummy matmuls over a zeroed
    # scratch tile so the real fp32 MLP starts warm (2 cycles/col instead
    # of 4). The scratch PSUM bank is never read.
    wsrc = nc.alloc_sbuf_tensor("wsrc", [128, 512], BF16)
    wps = nc.alloc_psum_tensor("wps", [128, 512], F32)
    nc.vector.memset(wsrc[:, :], 0.0)
    for _ in range(8):
        nc.tensor.matmul(wps[:, :], wsrc[:, 0:128], wsrc[:, :])

    # inputs resident before the first matmul (raw wait: the tile scheduler's
    # deadlock simulator doesn't model raw DMA increments, so this must
    # precede the TileContext). Only the tensor engine touches xws/wb
    # directly; every other consumer is downstream of a matmul.
    nc.tensor.wait_ge(isem, 32)

    with TileContext(nc) as tc:
        with (
            tc.tile_pool(name="sb", bufs=1) as sb,
            tc.tile_pool(name="ps", bufs=1, space="PSUM") as ps,
            tc.tile_pool(name="psm", bufs=2, space="PSUM") as psm,
        ):
            pA = ps.tile([64, 16 * BLK], F32, name="pA")
            h3 = sb.tile([64, AGENTS], F32)
            # h with trailing ones-row (bias fold); the const row arrives by
            # DMA while the MLP's first chunks are still in flight.
            h1a = sb.tile([33, AGENTS], F32)
            h2a = sb.tile([65, AGENTS], F32)
            nc.gpsimd.dma_start(out=h1a[32:33, :], in_=ones[:, :])
            nc.gpsimd.dma_start(out=h2a[64:65, :], in_=ones[:, :])

            p1 = {}
            p2 = {}
            p3 = {}

            def mlp_stage(c, layer):
                sl = slice(c * CHUNK, (c + 1) * CHUNK)
                if layer == 0:
                    p1[c] = psm.tile([32, CHUNK], F32, name=f"p1_{c}",
                                     tag="p1")
                    nc.tensor.matmul(p1[c], w1s_s, xws_s[:, sl])
                elif layer == 1:
                    nc.vector.tensor_scalar_max(h1a[0:32, sl], p1[c], 0.0)
                    p2[c] = psm.tile([64, CHUNK], F32, name=f"p2_{c}",
                                     tag="p23")
                    nc.tensor.matmul(p2[c], w2a_s, h1a[:, sl])
                elif layer == 2:
                    nc.scalar.activation(h2a[0:64, sl], p2[c],
                                         mybir.ActivationFunctionType.Relu)
                    p3[c] = psm.tile([64, CHUNK], F32, name=f"p3_{c}",
                                     tag="p23")
                    nc.tensor.matmul(p3[c], w3a_s, h2a[:, sl])
                else:
                    nc.scalar.activation(h3[:, sl], p3[c],
                                         mybir.ActivationFunctionType.Identity)
                    for b in range(c * 4, c * 4 + 4):
                        hsl = h3[:, b * BLK:(b + 1) * BLK]
                        nc.tensor.matmul(pA[:, b * BLK:(b + 1) * BLK],
                                         hsl, hsl)

            def softmax_q(q):
                qs = slice(q * CHUNK, (q + 1) * CHUNK)
                r_q = sb.tile([64, 4], F32, name=f"r{q}")
                nc.vector.reduce_max(
                    r_q, pA[:, qs].rearrange("p (b j) -> p b j", j=BLK),
                    axis=mybir.AxisListType.X)
                # G = exp(A - rowmax): per-partition, per-block shift via a
                # 0-stride broadcast along j
                rrep = bass.AP(tensor=r_q.tensor, offset=r_q.offset,
                               ap=[list(r_q.ap[0]), list(r_q.ap[1]),
                                   [0, BLK]])
                d_q = sb.tile([64, CHUNK], F32, name=f"d{q}")
                nc.vector.tensor_sub(
                    d_q.rearrange("p (b j) -> p b j", j=BLK),
                    pA[:, qs].rearrange("p (b j) -> p b j", j=BLK),
                    rrep)
                band_q = sb.tile([64, CHUNK], F32, name=f"bq{q}")
                nc.scalar.activation(band_q, d_q,
                                     mybir.ActivationFunctionType.Exp)
                nc.gpsimd.dma_start(out=bands[:, qs], in_=band_q)

            # software-pipelined emission: chunk c runs stage l while chunk
            # c+1 runs stage l-1; a chunk's softmax quarter follows its
            # attention immediately and overlaps later chunks' PE work.
            for step in range(NCH + 4):
                for c in range(NCH):
                    stage = step - c
                    if 0 <= stage <= 3:
                        mlp_stage(c, stage)
                    elif stage == 4:
                        softmax_q(c)

    nc.compile()
    return nc


def _get_nc():
    global _NC_CACHE
    if _NC_CACHE is None:
        _NC_CACHE = build_nc()
    return _NC_CACHE


def pack_inputs(xt_core, W1, b1, W2, b2, W3, b3):
    import ml_dtypes
    bf = ml_dtypes.bfloat16
    xT = xt_core.T.astype(np.float32)          # [2, 1024]
    x_hi = xT.astype(bf)
    x_lo = (xT - x_hi.astype(np.float32)).astype(bf)
    W1_hi = W1.astype(bf)
    W1_lo = (W1 - W1_hi.astype(np.float32)).astype(bf)
    b1_hi = b1.astype(bf)
    b1_lo = (b1 - b1_hi.astype(np.float32)).astype(bf)
    xws = np.zeros((10, XWS_COLS), dtype=bf)
    xws[0:2, :AGENTS] = x_hi
    xws[2:4, :AGENTS] = x_lo
    xws[4:6, :AGENTS] = x_hi
    xws[6:8, :AGENTS] = x_lo
    xws[8:10, :AGENTS] = np.ones((2, AGENTS), dtype=bf)
    xws[0:2, AGENTS:] = W1_hi
    xws[2:4, AGENTS:] = W1_hi
    xws[4:6, AGENTS:] = W1_lo
    xws[6:8, AGENTS:] = W1_lo
    xws[8, AGENTS:] = b1_hi
    xws[9, AGENTS:] = b1_lo
    wb = np.zeros((65, WB_COLS), dtype=np.float32)
    wb[0:64, 0:64] = W3
    wb[64, 0:64] = b3
    wb[0:32, 64:128] = W2
    wb[32, 64:128] = b2
    return xws, wb


def kernel(x, W1, b1, W2, b2, W3, b3, sub_batches, **run_kwargs):
    global LAST_RESULT
    x = np.asarray(x)
    xt = np.ascontiguousarray(x[:, -1, :], dtype=np.float32)  # [8192, 2]
    W1 = np.asarray(W1, dtype=np.float32)
    W2 = np.asarray(W2, dtype=np.float32)
    W3 = np.asarray(W3, dtype=np.float32)
    b1 = np.asarray(b1, dtype=np.float32)
    b2 = np.asarray(b2, dtype=np.float32)
    b3 = np.asarray(b3, dtype=np.float32)

    ones = np.ones((1, AGENTS), dtype=np.float32)
    in_maps = []
    for d in range(NCORES):
        xws, wb = pack_inputs(
            xt[d * AGENTS:(d + 1) * AGENTS, :], W1, b1, W2, b2, W3, b3)
        in_maps.append({"xws": xws, "wb": wb, "ones": ones})

    nc = _get_nc()
    res = run_bass_kernel_spmd(nc, in_maps, core_ids=list(range(NCORES)),
                               **run_kwargs)
    LAST_RESULT = res

    # Device ships G = exp(A - rowmax); the reference E = exp(A - m[j]) is
    # G^T per block (A symmetric). Paste each block transposed and apply the
    # row normalization E / (sum + eps) while assembling the zero canvas.
    full = np.zeros((BS, BS), dtype=np.float32)
    for d in range(NCORES):
        bd = np.asarray(res.results[d]["bands"])        # [64, 1024] = G
        for b in range(16):
            n = d * 16 + b                              # global 64-row block
            E = np.ascontiguousarray(bd[:, b * BLK:(b + 1) * BLK].T)
            P = E / (E.sum(axis=1, keepdims=True) + EPS)
            full[n * BLK:(n + 1) * BLK, n * BLK:(n + 1) * BLK] = P

    starts = np.asarray(sub_batches)[:, 0]
    canonical = np.array_equal(starts, np.arange(128, dtype=np.int64) * BLK)
    if not canonical:
        # General placement: extract the 64x64 blocks and scatter them at the
        # rows given by sub_batches (faithful to the reference .at[].set).
        scat = np.zeros((BS, BS), dtype=np.float32)
        for n in range(128):
            blk = full[n * BLK:(n + 1) * BLK, n * BLK:(n + 1) * BLK]
            rows = int(starts[n]) + np.arange(BLK)
            scat[np.ix_(rows, rows)] = blk
        full = scat
    return full


# revision 16
# speedup vs baseline: 1.0123x; 1.0123x over previous
"""Trainium2 Bass kernel for nn_AttentionCIDNN (block-diagonal crowd attention).

Problem: x[8192, 8, 2] -> last timestep -> 3-layer MLP -> h[8192, 64];
128 groups of 64 agents; per group A = h_g @ h_g^T, column-shifted softmax
P = exp(A - m[j]) / (sum_j exp(A - m[j]) + eps); scatter P onto the block
diagonal of an 8192 x 8192 zero matrix.

Sharding: 8 cores, each owns 1024 contiguous agents (16 groups). The output
is block-diagonal: only the 16 nonzero 64x64 blocks per core are computed.

Key algebra: A is bitwise symmetric on the PE (same contraction order for
[i,j] and [j,i]), and the reference's m[j] is the row-max, so
E = exp(A - m[j]) = G^T where G = exp(A - rowmax[i]) -- a PER-PARTITION
shift (cheap 0-stride broadcast) instead of a cross-partition one. The
device ships G; the host pastes each 64x64 block transposed and applies the
row normalization E/(sum+eps) during assembly.

Structure per core:
- two input DMAs on the gpsimd SWDGE queue, triggered raw right after the
  engine preamble; a dummy activation preloads the scalar ACT table during
  the DMA wait.
- all biases are folded into the matmuls via ones-row augmentation (host
  packs [x_hi;x_lo;x_hi;x_lo;1;1] against [W1_hi;W1_hi;W1_lo;W1_lo;b1_hi;
  b1_lo] for an exact bf16 L1; W2/W3 get ones-rows in h via a DMA'd const
  row), so the relus are bias-free and balance across vector/scalar.
- L2/L3/attention matmuls are true fp32: exp() amplifies any error in A
  (|A| up to ~168); bf16 or float32r anywhere in that chain pushes max rel
  err past the 2e-2 gate (measured 2.2e-2 with fp32r L2/L3).
- MLP in four 256-col chunks; each chunk's activations overlap the next
  chunk's matmuls; each chunk's 4 attention blocks and its softmax quarter
  (rowmax -> subtract -> exp -> DMA out) follow immediately and overlap
  later chunks' PE work.

Self-contained: hardcodes all shapes; builds the Bass graph once per process.
"""

import os
os.environ.setdefault("JAX_PLATFORMS", "axon")  # device exec path under axon

import numpy as np

import concourse.bass as bass
import concourse.bacc as bacc
import concourse.mybir as mybir
from concourse.tile import TileContext
from concourse.bass_utils import run_bass_kernel_spmd

F32 = mybir.dt.float32
F32R = mybir.dt.float32r
BF16 = mybir.dt.bfloat16

BS = 8192          # total agents
NCORES = 8
AGENTS = BS // NCORES   # 1024 agents per core
BLK = 64                # agents per attention group
EPS = np.float32(1e-7)
NCH = 4
CHUNK = AGENTS // NCH   # 256: MLP chunk = softmax quarter = 4 blocks

# xws (bf16): [10, 1056] = K-stacked exact-f32 split of [xT; 1] against
#   [W1; b1]: rows 0:2 x_hi, 2:4 x_lo, 4:6 x_hi, 6:8 x_lo, 8 ones, 9 ones
#   paired with w1s rows W1_hi, W1_hi, W1_lo, W1_lo, b1_hi, b1_lo.
XWS_COLS = AGENTS + 32
# wb (f32): [65, 128] = W3a [65, 0:64] | W2a rows 0:33 [64:128]
#   (Wka = [Wk; bk^T], consumed against h with a trailing ones-row)
WB_COLS = 128

_NC_CACHE = None
LAST_RESULT = None  # BassKernelResults of the most recent run (for test harness)


def build_nc():
    """Build the single-core Bass graph (identical on all 8 cores)."""
    nc = bacc.Bacc("TRN2", target_bir_lowering=False)

    xws = nc.declare_dram_parameter("xws", [10, XWS_COLS], BF16,
                                    isOutput=False)
    wb = nc.declare_dram_parameter("wb", [65, WB_COLS], F32, isOutput=False)
    ones = nc.declare_dram_parameter("ones", [1, AGENTS], F32, isOutput=False)
    bands = nc.declare_dram_parameter("bands", [64, 16 * BLK], F32,
                                      isOutput=True)

    # ---- input DMAs on the gpsimd SWDGE queue, emitted raw so they trigger
    # right after the engine preamble instead of behind the tile-pool entry.
    isem = nc.alloc_semaphore("inp")
    xws_s = nc.alloc_sbuf_tensor("xws_s", [10, XWS_COLS], BF16)
    wb_s = nc.alloc_sbuf_tensor("wb_s", [65, WB_COLS], F32)
    nc.gpsimd.dma_start(out=xws_s[:, :], in_=xws[:, :]).then_inc(isem, 16)
    nc.gpsimd.dma_start(out=wb_s[:, :], in_=wb[:, :]).then_inc(isem, 16)

    w1s_s = xws_s[:, AGENTS:AGENTS + 32]
    w3a_s = wb_s[0:65, 0:64]
    w2a_s = wb_s[0:33, 64:128]

    # scalar: preload the ACT table (1.3us) while the input DMAs fly; the
    # scratch tile is uninitialized, the result is never read.
    scr = nc.alloc_sbuf_tensor("scr", [1, 8], F32)
    scr2 = nc.alloc_sbuf_tensor("scr2", [1, 8], F32)
    nc.scalar.activation(scr2[:, :], scr[:, :],
                         mybir.ActivationFunctionType.Relu)

    # PE warm-up: the PE runs at 1.2 GHz until its free-running activity
    # window sees ~3.4us of continuous matmul traffic, then doubles to
    # 2.4 GHz. Burn the input-DMA wait on dummy matmuls over a zeroed
    # scratch tile so the real fp32 MLP starts warm (2 cycles/col instead
    # of 4). The scratch PSUM bank is never read.
    wsrc = nc.alloc_sbuf_tensor("wsrc", [128, 512], BF16)
    wps = nc.alloc_psum_tensor("wps", [128, 512], F32)
    nc.vector.memset(wsrc[:, :], 0.0)
    for _ in range(12):
        nc.tensor.matmul(wps[:, :], wsrc[:, 0:128], wsrc[:, :])

    # inputs resident before the first matmul (raw wait: the tile scheduler's
    # deadlock simulator doesn't model raw DMA increments, so this must
    # precede the TileContext). Only the tensor engine touches xws/wb
    # directly; every other consumer is downstream of a matmul.
    nc.tensor.wait_ge(isem, 32)

    with TileContext(nc) as tc:
        with (
            tc.tile_pool(name="sb", bufs=1) as sb,
            tc.tile_pool(name="ps", bufs=1, space="PSUM") as ps,
            tc.tile_pool(name="psm", bufs=2, space="PSUM") as psm,
        ):
            pA = ps.tile([64, 16 * BLK], F32, name="pA")
            h3 = sb.tile([64, AGENTS], F32)
            # h with trailing ones-row (bias fold); the const row arrives by
            # DMA while the MLP's first chunks are still in flight.
            h1a = sb.tile([33, AGENTS], F32)
            h2a = sb.tile([65, AGENTS], F32)
            nc.gpsimd.dma_start(out=h1a[32:33, :], in_=ones[:, :])
            nc.gpsimd.dma_start(out=h2a[64:65, :], in_=ones[:, :])

            p1 = {}
            p2 = {}
            p3 = {}

            def mlp_stage(c, layer):
                sl = slice(c * CHUNK, (c + 1) * CHUNK)
                if layer == 0:
                    p1[c] = psm.tile([32, CHUNK], F32, name=f"p1_{c}",
                                     tag="p1")
                    nc.tensor.matmul(p1[c], w1s_s, xws_s[:, sl])
                elif layer == 1:
                    nc.vector.tensor_scalar_max(h1a[0:32, sl], p1[c], 0.0)
                    p2[c] = psm.tile([64, CHUNK], F32, name=f"p2_{c}",
                                     tag="p23")
                    nc.tensor.matmul(p2[c], w2a_s, h1a[:, sl])
                elif layer == 2:
                    nc.scalar.activation(h2a[0:64, sl], p2[c],
                                         mybir.ActivationFunctionType.Relu)
                    p3[c] = psm.tile([64, CHUNK], F32, name=f"p3_{c}",
                                     tag="p23")
                    nc.tensor.matmul(p3[c], w3a_s, h2a[:, sl])
                else:
                    # last chunk's h3-copy on vector: scalar must be free
                    # for the tail-critical exp of the final quarter
                    if c == NCH - 1:
                        nc.vector.tensor_copy(h3[:, sl], p3[c])
                    else:
                        nc.scalar.activation(
                            h3[:, sl], p3[c],
                            mybir.ActivationFunctionType.Identity)
                    for b in range(c * 4, c * 4 + 4):
                        hsl = h3[:, b * BLK:(b + 1) * BLK]
                        nc.tensor.matmul(pA[:, b * BLK:(b + 1) * BLK],
                                         hsl, hsl)

            def softmax_q(q):
                qs = slice(q * CHUNK, (q + 1) * CHUNK)
                r_q = sb.tile([64, 4], F32, name=f"r{q}")
                nc.vector.reduce_max(
                    r_q, pA[:, qs].rearrange("p (b j) -> p b j", j=BLK),
                    axis=mybir.AxisListType.X)
                # G = exp(A - rowmax): per-partition, per-block shift via a
                # 0-stride broadcast along j
                rrep = bass.AP(tensor=r_q.tensor, offset=r_q.offset,
                               ap=[list(r_q.ap[0]), list(r_q.ap[1]),
                                   [0, BLK]])
                d_q = sb.tile([64, CHUNK], F32, name=f"d{q}")
                nc.vector.tensor_sub(
                    d_q.rearrange("p (b j) -> p b j", j=BLK),
                    pA[:, qs].rearrange("p (b j) -> p b j", j=BLK),
                    rrep)
                band_q = sb.tile([64, CHUNK], F32, name=f"bq{q}")
                nc.scalar.activation(band_q, d_q,
                                     mybir.ActivationFunctionType.Exp)
                nc.gpsimd.dma_start(out=bands[:, qs], in_=band_q)

            # software-pipelined emission: chunk c runs stage l while chunk
            # c+1 runs stage l-1; a chunk's softmax quarter follows its
            # attention immediately and overlaps later chunks' PE work.
            for step in range(NCH + 4):
                for c in range(NCH):
                    stage = step - c
                    if 0 <= stage <= 3:
                        mlp_stage(c, stage)
                    elif stage == 4:
                        softmax_q(c)

    nc.compile()
    return nc


def _get_nc():
    global _NC_CACHE
    if _NC_CACHE is None:
        _NC_CACHE = build_nc()
    return _NC_CACHE


def pack_inputs(xt_core, W1, b1, W2, b2, W3, b3):
    import ml_dtypes
    bf = ml_dtypes.bfloat16
    xT = xt_core.T.astype(np.float32)          # [2, 1024]
    x_hi = xT.astype(bf)
    x_lo = (xT - x_hi.astype(np.float32)).astype(bf)
    W1_hi = W1.astype(bf)
    W1_lo = (W1 - W1_hi.astype(np.float32)).astype(bf)
    b1_hi = b1.astype(bf)
    b1_lo = (b1 - b1_hi.astype(np.float32)).astype(bf)
    xws = np.zeros((10, XWS_COLS), dtype=bf)
    xws[0:2, :AGENTS] = x_hi
    xws[2:4, :AGENTS] = x_lo
    xws[4:6, :AGENTS] = x_hi
    xws[6:8, :AGENTS] = x_lo
    xws[8:10, :AGENTS] = np.ones((2, AGENTS), dtype=bf)
    xws[0:2, AGENTS:] = W1_hi
    xws[2:4, AGENTS:] = W1_hi
    xws[4:6, AGENTS:] = W1_lo
    xws[6:8, AGENTS:] = W1_lo
    xws[8, AGENTS:] = b1_hi
    xws[9, AGENTS:] = b1_lo
    wb = np.zeros((65, WB_COLS), dtype=np.float32)
    wb[0:64, 0:64] = W3
    wb[64, 0:64] = b3
    wb[0:32, 64:128] = W2
    wb[32, 64:128] = b2
    return xws, wb


def kernel(x, W1, b1, W2, b2, W3, b3, sub_batches, **run_kwargs):
    global LAST_RESULT
    x = np.asarray(x)
    xt = np.ascontiguousarray(x[:, -1, :], dtype=np.float32)  # [8192, 2]
    W1 = np.asarray(W1, dtype=np.float32)
    W2 = np.asarray(W2, dtype=np.float32)
    W3 = np.asarray(W3, dtype=np.float32)
    b1 = np.asarray(b1, dtype=np.float32)
    b2 = np.asarray(b2, dtype=np.float32)
    b3 = np.asarray(b3, dtype=np.float32)

    ones = np.ones((1, AGENTS), dtype=np.float32)
    in_maps = []
    for d in range(NCORES):
        xws, wb = pack_inputs(
            xt[d * AGENTS:(d + 1) * AGENTS, :], W1, b1, W2, b2, W3, b3)
        in_maps.append({"xws": xws, "wb": wb, "ones": ones})

    nc = _get_nc()
    res = run_bass_kernel_spmd(nc, in_maps, core_ids=list(range(NCORES)),
                               **run_kwargs)
    LAST_RESULT = res

    # Device ships G = exp(A - rowmax); the reference E = exp(A - m[j]) is
    # G^T per block (A symmetric). Paste each block transposed and apply the
    # row normalization E / (sum + eps) while assembling the zero canvas.
    full = np.zeros((BS, BS), dtype=np.float32)
    for d in range(NCORES):
        bd = np.asarray(res.results[d]["bands"])        # [64, 1024] = G
        for b in range(16):
            n = d * 16 + b                              # global 64-row block
            E = np.ascontiguousarray(bd[:, b * BLK:(b + 1) * BLK].T)
            P = E / (E.sum(axis=1, keepdims=True) + EPS)
            full[n * BLK:(n + 1) * BLK, n * BLK:(n + 1) * BLK] = P

    starts = np.asarray(sub_batches)[:, 0]
    canonical = np.array_equal(starts, np.arange(128, dtype=np.int64) * BLK)
    if not canonical:
        # General placement: extract the 64x64 blocks and scatter them at the
        # rows given by sub_batches (faithful to the reference .at[].set).
        scat = np.zeros((BS, BS), dtype=np.float32)
        for n in range(128):
            blk = full[n * BLK:(n + 1) * BLK, n * BLK:(n + 1) * BLK]
            rows = int(starts[n]) + np.arange(BLK)
            scat[np.ix_(rows, rows)] = blk
        full = scat
    return full


# revision 18
# speedup vs baseline: 1.0288x; 1.0163x over previous
"""Trainium2 Bass kernel for nn_AttentionCIDNN (block-diagonal crowd attention).

Problem: x[8192, 8, 2] -> last timestep -> 3-layer MLP -> h[8192, 64];
128 groups of 64 agents; per group A = h_g @ h_g^T, column-shifted softmax
P = exp(A - m[j]) / (sum_j exp(A - m[j]) + eps); scatter P onto the block
diagonal of an 8192 x 8192 zero matrix.

Sharding: 8 cores, each owns 1024 contiguous agents (16 groups). The output
is block-diagonal: only the 16 nonzero 64x64 blocks per core are computed.

Key algebra: A is bitwise symmetric on the PE (same contraction order for
[i,j] and [j,i]), and the reference's m[j] is the row-max, so
E = exp(A - m[j]) = G^T where G = exp(A - rowmax[i]) -- a PER-PARTITION
shift (cheap 0-stride broadcast) instead of a cross-partition one. The
device ships G; the host pastes each 64x64 block transposed and applies the
row normalization E/(sum+eps) during assembly.

Structure per core:
- two input DMAs on the gpsimd SWDGE queue, triggered raw right after the
  engine preamble; a dummy activation preloads the scalar ACT table during
  the DMA wait.
- all biases are folded into the matmuls via ones-row augmentation (host
  packs [x_hi;x_lo;x_hi;x_lo;1;1] against [W1_hi;W1_hi;W1_lo;W1_lo;b1_hi;
  b1_lo] for an exact bf16 L1; W2/W3 get ones-rows in h via a DMA'd const
  row), so the relus are bias-free and balance across vector/scalar.
- L2/L3/attention matmuls are true fp32: exp() amplifies any error in A
  (|A| up to ~168); bf16 or float32r anywhere in that chain pushes max rel
  err past the 2e-2 gate (measured 2.2e-2 with fp32r L2/L3).
- MLP in four 256-col chunks; each chunk's activations overlap the next
  chunk's matmuls; each chunk's 4 attention blocks and its softmax quarter
  (rowmax -> subtract -> exp -> DMA out) follow immediately and overlap
  later chunks' PE work.

Self-contained: hardcodes all shapes; builds the Bass graph once per process.
"""

import os
os.environ.setdefault("JAX_PLATFORMS", "axon")  # device exec path under axon

import numpy as np

import concourse.bass as bass
import concourse.bacc as bacc
import concourse.mybir as mybir
from concourse.tile import TileContext
from concourse.bass_utils import run_bass_kernel_spmd

F32 = mybir.dt.float32
F32R = mybir.dt.float32r
BF16 = mybir.dt.bfloat16

BS = 8192          # total agents
NCORES = 8
AGENTS = BS // NCORES   # 1024 agents per core
BLK = 64                # agents per attention group
EPS = np.float32(1e-7)
NCH = 4
CHUNK = AGENTS // NCH   # 256: MLP chunk = softmax quarter = 4 blocks

# xws (bf16): [10, 1056] = K-stacked exact-f32 split of [xT; 1] against
#   [W1; b1]: rows 0:2 x_hi, 2:4 x_lo, 4:6 x_hi, 6:8 x_lo, 8 ones, 9 ones
#   paired with w1s rows W1_hi, W1_hi, W1_lo, W1_lo, b1_hi, b1_lo.
XWS_COLS = AGENTS + 32
# wb (f32): [65, 128] = W3a [65, 0:64] | W2a rows 0:33 [64:128]
#   (Wka = [Wk; bk^T], consumed against h with a trailing ones-row)
WB_COLS = 128

_NC_CACHE = None
LAST_RESULT = None  # BassKernelResults of the most recent run (for test harness)


def build_nc():
    """Build the single-core Bass graph (identical on all 8 cores)."""
    nc = bacc.Bacc("TRN2", target_bir_lowering=False)

    xws = nc.declare_dram_parameter("xws", [10, XWS_COLS], BF16,
                                    isOutput=False)
    wb = nc.declare_dram_parameter("wb", [65, WB_COLS], F32, isOutput=False)
    ones = nc.declare_dram_parameter("ones", [1, AGENTS], F32, isOutput=False)
    bands = nc.declare_dram_parameter("bands", [64, 16 * BLK], F32,
                                      isOutput=True)

    # ---- input DMAs on the gpsimd SWDGE queue, emitted raw so they trigger
    # right after the engine preamble instead of behind the tile-pool entry.
    isem = nc.alloc_semaphore("inp")
    xws_s = nc.alloc_sbuf_tensor("xws_s", [10, XWS_COLS], BF16)
    wb_s = nc.alloc_sbuf_tensor("wb_s", [65, WB_COLS], F32)
    nc.gpsimd.dma_start(out=xws_s[:, :], in_=xws[:, :]).then_inc(isem, 16)
    nc.gpsimd.dma_start(out=wb_s[:, :], in_=wb[:, :]).then_inc(isem, 16)

    w1s_s = xws_s[:, AGENTS:AGENTS + 32]
    w3a_s = wb_s[0:65, 0:64]
    w2a_s = wb_s[0:33, 64:128]

    # scalar: preload the ACT table (1.3us) while the input DMAs fly; the
    # scratch tile is uninitialized, the result is never read.
    scr = nc.alloc_sbuf_tensor("scr", [1, 8], F32)
    scr2 = nc.alloc_sbuf_tensor("scr2", [1, 8], F32)
    nc.scalar.activation(scr2[:, :], scr[:, :],
                         mybir.ActivationFunctionType.Relu)

    # PE warm-up: the PE runs at 1.2 GHz until its free-running activity
    # window sees ~3.4us of continuous matmul traffic, then doubles to
    # 2.4 GHz. Burn the input-DMA wait on dummy matmuls over a zeroed
    # scratch tile so the real fp32 MLP starts warm (2 cycles/col instead
    # of 4). The scratch PSUM bank is never read.
    wsrc = nc.alloc_sbuf_tensor("wsrc", [128, 512], BF16)
    wps = nc.alloc_psum_tensor("wps", [128, 512], F32)
    nc.vector.memset(wsrc[:, :], 0.0)
    for _ in range(10):
        nc.tensor.matmul(wps[:, :], wsrc[:, 0:128], wsrc[:, :])

    # inputs resident before the first matmul (raw wait: the tile scheduler's
    # deadlock simulator doesn't model raw DMA increments, so this must
    # precede the TileContext). Only the tensor engine touches xws/wb
    # directly; every other consumer is downstream of a matmul.
    nc.tensor.wait_ge(isem, 32)

    with TileContext(nc) as tc:
        with (
            tc.tile_pool(name="sb", bufs=1) as sb,
            tc.tile_pool(name="ps", bufs=1, space="PSUM") as ps,
            tc.tile_pool(name="psm", bufs=2, space="PSUM") as psm,
        ):
            pA = ps.tile([64, 16 * BLK], F32, name="pA")
            h3 = sb.tile([64, AGENTS], F32)
            # h with trailing ones-row (bias fold); the const row arrives by
            # DMA while the MLP's first chunks are still in flight.
            h1a = sb.tile([33, AGENTS], F32)
            h2a = sb.tile([65, AGENTS], F32)
            nc.gpsimd.dma_start(out=h1a[32:33, :], in_=ones[:, :])
            nc.gpsimd.dma_start(out=h2a[64:65, :], in_=ones[:, :])

            # MLP in 2 chunks of 512: big enough per-stage work to hide the
            # ~0.5us cross-engine semaphore-hop latency that dominated with
            # 256-col chunks. h3-copy of chunk 1 runs on VECTOR so it can
            # proceed during attention blocks 0-7 (scalar is mid relu/exp).
            MC = AGENTS // 2
            p1 = {}
            p2 = {}
            p3 = {}
            for c in range(2):
                sl = slice(c * MC, (c + 1) * MC)
                p1[c] = psm.tile([32, MC], F32, name=f"p1_{c}", tag="p1")
                nc.tensor.matmul(p1[c], w1s_s, xws_s[:, sl])
            for c in range(2):
                sl = slice(c * MC, (c + 1) * MC)
                nc.vector.tensor_scalar_max(h1a[0:32, sl], p1[c], 0.0)
                p2[c] = psm.tile([64, MC], F32, name=f"p2_{c}", tag="p23")
                nc.tensor.matmul(p2[c], w2a_s, h1a[:, sl])
            for c in range(2):
                sl = slice(c * MC, (c + 1) * MC)
                nc.scalar.activation(h2a[0:64, sl], p2[c],
                                     mybir.ActivationFunctionType.Relu)
                p3[c] = psm.tile([64, MC], F32, name=f"p3_{c}", tag="p23")
                nc.tensor.matmul(p3[c], w3a_s, h2a[:, sl])

            nc.scalar.activation(h3[:, 0:MC], p3[0],
                                 mybir.ActivationFunctionType.Identity)
            nc.vector.tensor_copy(h3[:, MC:AGENTS], p3[1])
            for b in range(16):
                hsl = h3[:, b * BLK:(b + 1) * BLK]
                nc.tensor.matmul(pA[:, b * BLK:(b + 1) * BLK], hsl, hsl)

            # softmax quarters overlap later attention blocks; quarter q
            # only depends on pA columns of its own 4 blocks.
            for q in range(4):
                qs = slice(q * CHUNK, (q + 1) * CHUNK)
                r_q = sb.tile([64, 4], F32, name=f"r{q}")
                nc.vector.reduce_max(
                    r_q, pA[:, qs].rearrange("p (b j) -> p b j", j=BLK),
                    axis=mybir.AxisListType.X)
                # G = exp(A - rowmax): per-partition, per-block shift via a
                # 0-stride broadcast along j
                rrep = bass.AP(tensor=r_q.tensor, offset=r_q.offset,
                               ap=[list(r_q.ap[0]), list(r_q.ap[1]),
                                   [0, BLK]])
                d_q = sb.tile([64, CHUNK], F32, name=f"d{q}")
                nc.vector.tensor_sub(
                    d_q.rearrange("p (b j) -> p b j", j=BLK),
                    pA[:, qs].rearrange("p (b j) -> p b j", j=BLK),
                    rrep)
                band_q = sb.tile([64, CHUNK], F32, name=f"bq{q}")
                nc.scalar.activation(band_q, d_q,
                                     mybir.ActivationFunctionType.Exp)
                nc.gpsimd.dma_start(out=bands[:, qs], in_=band_q)

    nc.compile()
    return nc


def _get_nc():
    global _NC_CACHE
    if _NC_CACHE is None:
        _NC_CACHE = build_nc()
    return _NC_CACHE


def pack_inputs(xt_core, W1, b1, W2, b2, W3, b3):
    import ml_dtypes
    bf = ml_dtypes.bfloat16
    xT = xt_core.T.astype(np.float32)          # [2, 1024]
    x_hi = xT.astype(bf)
    x_lo = (xT - x_hi.astype(np.float32)).astype(bf)
    W1_hi = W1.astype(bf)
    W1_lo = (W1 - W1_hi.astype(np.float32)).astype(bf)
    b1_hi = b1.astype(bf)
    b1_lo = (b1 - b1_hi.astype(np.float32)).astype(bf)
    xws = np.zeros((10, XWS_COLS), dtype=bf)
    xws[0:2, :AGENTS] = x_hi
    xws[2:4, :AGENTS] = x_lo
    xws[4:6, :AGENTS] = x_hi
    xws[6:8, :AGENTS] = x_lo
    xws[8:10, :AGENTS] = np.ones((2, AGENTS), dtype=bf)
    xws[0:2, AGENTS:] = W1_hi
    xws[2:4, AGENTS:] = W1_hi
    xws[4:6, AGENTS:] = W1_lo
    xws[6:8, AGENTS:] = W1_lo
    xws[8, AGENTS:] = b1_hi
    xws[9, AGENTS:] = b1_lo
    wb = np.zeros((65, WB_COLS), dtype=np.float32)
    wb[0:64, 0:64] = W3
    wb[64, 0:64] = b3
    wb[0:32, 64:128] = W2
    wb[32, 64:128] = b2
    return xws, wb


def kernel(x, W1, b1, W2, b2, W3, b3, sub_batches, **run_kwargs):
    global LAST_RESULT
    x = np.asarray(x)
    xt = np.ascontiguousarray(x[:, -1, :], dtype=np.float32)  # [8192, 2]
    W1 = np.asarray(W1, dtype=np.float32)
    W2 = np.asarray(W2, dtype=np.float32)
    W3 = np.asarray(W3, dtype=np.float32)
    b1 = np.asarray(b1, dtype=np.float32)
    b2 = np.asarray(b2, dtype=np.float32)
    b3 = np.asarray(b3, dtype=np.float32)

    ones = np.ones((1, AGENTS), dtype=np.float32)
    in_maps = []
    for d in range(NCORES):
        xws, wb = pack_inputs(
            xt[d * AGENTS:(d + 1) * AGENTS, :], W1, b1, W2, b2, W3, b3)
        in_maps.append({"xws": xws, "wb": wb, "ones": ones})

    nc = _get_nc()
    res = run_bass_kernel_spmd(nc, in_maps, core_ids=list(range(NCORES)),
                               **run_kwargs)
    LAST_RESULT = res

    # Device ships G = exp(A - rowmax); the reference E = exp(A - m[j]) is
    # G^T per block (A symmetric). Paste each block transposed and apply the
    # row normalization E / (sum + eps) while assembling the zero canvas.
    full = np.zeros((BS, BS), dtype=np.float32)
    for d in range(NCORES):
        bd = np.asarray(res.results[d]["bands"])        # [64, 1024] = G
        for b in range(16):
            n = d * 16 + b                              # global 64-row block
            E = np.ascontiguousarray(bd[:, b * BLK:(b + 1) * BLK].T)
            P = E / (E.sum(axis=1, keepdims=True) + EPS)
            full[n * BLK:(n + 1) * BLK, n * BLK:(n + 1) * BLK] = P

    starts = np.asarray(sub_batches)[:, 0]
    canonical = np.array_equal(starts, np.arange(128, dtype=np.int64) * BLK)
    if not canonical:
        # General placement: extract the 64x64 blocks and scatter them at the
        # rows given by sub_batches (faithful to the reference .at[].set).
        scat = np.zeros((BS, BS), dtype=np.float32)
        for n in range(128):
            blk = full[n * BLK:(n + 1) * BLK, n * BLK:(n + 1) * BLK]
            rows = int(starts[n]) + np.arange(BLK)
            scat[np.ix_(rows, rows)] = blk
        full = scat
    return full


# revision 19
# speedup vs baseline: 1.0718x; 1.0418x over previous
"""Trainium2 Bass kernel for nn_AttentionCIDNN (block-diagonal crowd attention).

Problem: x[8192, 8, 2] -> last timestep -> 3-layer MLP -> h[8192, 64];
128 groups of 64 agents; per group A = h_g @ h_g^T, column-shifted softmax
P = exp(A - m[j]) / (sum_j exp(A - m[j]) + eps); scatter P onto the block
diagonal of an 8192 x 8192 zero matrix.

Sharding: 8 cores, each owns 1024 contiguous agents (16 groups). The output
is block-diagonal: only the 16 nonzero 64x64 blocks per core are computed.

Key algebra: A is bitwise symmetric on the PE (same contraction order for
[i,j] and [j,i]), and the reference's m[j] is the row-max, so
E = exp(A - m[j]) = G^T where G = exp(A - rowmax[i]) -- a PER-PARTITION
shift (cheap 0-stride broadcast) instead of a cross-partition one. The
device ships G; the host pastes each 64x64 block transposed and applies the
row normalization E/(sum+eps) during assembly.

Structure per core:
- two input DMAs on the gpsimd SWDGE queue, triggered raw right after the
  engine preamble; a dummy activation preloads the scalar ACT table during
  the DMA wait.
- all biases are folded into the matmuls via ones-row augmentation (host
  packs [x_hi;x_lo;x_hi;x_lo;1;1] against [W1_hi;W1_hi;W1_lo;W1_lo;b1_hi;
  b1_lo] for an exact bf16 L1; W2/W3 get ones-rows in h via a DMA'd const
  row), so the relus are bias-free and balance across vector/scalar.
- L2/L3/attention matmuls are true fp32: exp() amplifies any error in A
  (|A| up to ~168); bf16 or float32r anywhere in that chain pushes max rel
  err past the 2e-2 gate (measured 2.2e-2 with fp32r L2/L3).
- MLP in four 256-col chunks; each chunk's activations overlap the next
  chunk's matmuls; each chunk's 4 attention blocks and its softmax quarter
  (rowmax -> subtract -> exp -> DMA out) follow immediately and overlap
  later chunks' PE work.

Self-contained: hardcodes all shapes; builds the Bass graph once per process.
"""

import os
os.environ.setdefault("JAX_PLATFORMS", "axon")  # device exec path under axon

import numpy as np

import concourse.bass as bass
import concourse.bacc as bacc
import concourse.mybir as mybir
from concourse.tile import TileContext
from concourse.bass_utils import run_bass_kernel_spmd

F32 = mybir.dt.float32
F32R = mybir.dt.float32r
BF16 = mybir.dt.bfloat16

BS = 8192          # total agents
NCORES = 8
AGENTS = BS // NCORES   # 1024 agents per core
BLK = 64                # agents per attention group
EPS = np.float32(1e-7)
NCH = 4
CHUNK = AGENTS // NCH   # 256: MLP chunk = softmax quarter = 4 blocks

# xws (bf16): [10, 1056] = K-stacked exact-f32 split of [xT; 1] against
#   [W1; b1]: rows 0:2 x_hi, 2:4 x_lo, 4:6 x_hi, 6:8 x_lo, 8 ones, 9 ones
#   paired with w1s rows W1_hi, W1_hi, W1_lo, W1_lo, b1_hi, b1_lo.
XWS_COLS = AGENTS + 32
# wb (f32): [65, 128] = W3a [65, 0:64] | W2a rows 0:33 [64:128]
#   (Wka = [Wk; bk^T], consumed against h with a trailing ones-row)
WB_COLS = 128

_NC_CACHE = None
LAST_RESULT = None  # BassKernelResults of the most recent run (for test harness)


def build_nc():
    """Build the single-core Bass graph (identical on all 8 cores)."""
    nc = bacc.Bacc("TRN2", target_bir_lowering=False)

    xws = nc.declare_dram_parameter("xws", [10, XWS_COLS], BF16,
                                    isOutput=False)
    wb = nc.declare_dram_parameter("wb", [65, WB_COLS], F32, isOutput=False)
    ones = nc.declare_dram_parameter("ones", [1, AGENTS], F32, isOutput=False)
    bands = nc.declare_dram_parameter("bands", [64, 16 * BLK], F32,
                                      isOutput=True)

    # ---- input DMAs on the gpsimd SWDGE queue, emitted raw so they trigger
    # right after the engine preamble instead of behind the tile-pool entry.
    isem = nc.alloc_semaphore("inp")
    xws_s = nc.alloc_sbuf_tensor("xws_s", [10, XWS_COLS], BF16)
    wb_s = nc.alloc_sbuf_tensor("wb_s", [65, WB_COLS], F32)
    nc.gpsimd.dma_start(out=xws_s[:, :], in_=xws[:, :]).then_inc(isem, 16)
    nc.gpsimd.dma_start(out=wb_s[:, :], in_=wb[:, :]).then_inc(isem, 16)

    w1s_s = xws_s[:, AGENTS:AGENTS + 32]
    w3a_s = wb_s[0:65, 0:64]
    w2a_s = wb_s[0:33, 64:128]

    # scalar: preload the ACT table (1.3us) while the input DMAs fly; the
    # scratch tile is uninitialized, the result is never read.
    scr = nc.alloc_sbuf_tensor("scr", [1, 8], F32)
    scr2 = nc.alloc_sbuf_tensor("scr2", [1, 8], F32)
    nc.scalar.activation(scr2[:, :], scr[:, :],
                         mybir.ActivationFunctionType.Relu)

    # PE warm-up: the PE runs at 1.2 GHz until its free-running activity
    # window sees ~3.4us of continuous matmul traffic, then doubles to
    # 2.4 GHz. Burn the input-DMA wait on dummy matmuls over a zeroed
    # scratch tile so the real fp32 MLP starts warm (2 cycles/col instead
    # of 4). The scratch PSUM bank is never read.
    wsrc = nc.alloc_sbuf_tensor("wsrc", [128, 512], BF16)
    wps = nc.alloc_psum_tensor("wps", [128, 512], F32)
    nc.vector.memset(wsrc[:, :], 0.0)
    for _ in range(8):
        nc.tensor.matmul(wps[:, :], wsrc[:, 0:128], wsrc[:, :])
    for _ in range(8):
        nc.tensor.matmul(wps[:, 0:64], wsrc[:, 0:128], wsrc[:, 0:64])

    # inputs resident before the first matmul (raw wait: the tile scheduler's
    # deadlock simulator doesn't model raw DMA increments, so this must
    # precede the TileContext). Only the tensor engine touches xws/wb
    # directly; every other consumer is downstream of a matmul.
    nc.tensor.wait_ge(isem, 32)

    with TileContext(nc) as tc:
        with (
            tc.tile_pool(name="sb", bufs=1) as sb,
            tc.tile_pool(name="ps", bufs=1, space="PSUM") as ps,
            tc.tile_pool(name="psm", bufs=2, space="PSUM") as psm,
        ):
            pA0 = ps.tile([64, 8 * BLK], F32, name="pA0")
            pA1 = ps.tile([64, 8 * BLK], F32, name="pA1")
            h3 = sb.tile([64, AGENTS], F32)
            # h with trailing ones-row (bias fold); the const row arrives by
            # DMA while the MLP's first chunks are still in flight.
            h1a = sb.tile([33, AGENTS], F32)
            h2a = sb.tile([65, AGENTS], F32)
            nc.gpsimd.dma_start(out=h1a[32:33, :], in_=ones[:, :])
            nc.gpsimd.dma_start(out=h2a[64:65, :], in_=ones[:, :])

            # MLP in 2 chunks of 512: big enough per-stage work to hide the
            # ~0.5us cross-engine semaphore-hop latency that dominated with
            # 256-col chunks. h3-copy of chunk 1 runs on VECTOR so it can
            # proceed during attention blocks 0-7 (scalar is mid relu/exp).
            MC = AGENTS // 2
            p1 = {}
            p2 = {}
            p3 = {}
            for c in range(2):
                sl = slice(c * MC, (c + 1) * MC)
                p1[c] = psm.tile([32, MC], F32, name=f"p1_{c}", tag="p1")
                nc.tensor.matmul(p1[c], w1s_s, xws_s[:, sl])
            for c in range(2):
                sl = slice(c * MC, (c + 1) * MC)
                nc.vector.tensor_scalar_max(h1a[0:32, sl], p1[c], 0.0)
                p2[c] = psm.tile([64, MC], F32, name=f"p2_{c}", tag="p23")
                nc.tensor.matmul(p2[c], w2a_s, h1a[:, sl])
            for c in range(2):
                sl = slice(c * MC, (c + 1) * MC)
                nc.scalar.activation(h2a[0:64, sl], p2[c],
                                     mybir.ActivationFunctionType.Relu)
                p3[c] = psm.tile([64, MC], F32, name=f"p3_{c}", tag="p23")
                nc.tensor.matmul(p3[c], w3a_s, h2a[:, sl])

            nc.scalar.activation(h3[:, 0:MC], p3[0],
                                 mybir.ActivationFunctionType.Identity)
            nc.vector.tensor_copy(h3[:, MC:AGENTS], p3[1])
            for b in range(16):
                hsl = h3[:, b * BLK:(b + 1) * BLK]
                pa = pA0 if b < 8 else pA1
                nc.tensor.matmul(pa[:, (b % 8) * BLK:(b % 8 + 1) * BLK],
                                 hsl, hsl)

            # softmax quarters overlap later attention blocks; quarter q
            # only depends on pA columns of its own 4 blocks.
            for q in range(4):
                qs = slice(q * CHUNK, (q + 1) * CHUNK)
                pa = (pA0 if q < 2 else pA1)[:, (q % 2) * CHUNK:
                                             (q % 2 + 1) * CHUNK]
                r_q = sb.tile([64, 4], F32, name=f"r{q}")
                nc.vector.reduce_max(
                    r_q, pa.rearrange("p (b j) -> p b j", j=BLK),
                    axis=mybir.AxisListType.X)
                # G = exp(A - rowmax): per-partition, per-block shift via a
                # 0-stride broadcast along j
                rrep = bass.AP(tensor=r_q.tensor, offset=r_q.offset,
                               ap=[list(r_q.ap[0]), list(r_q.ap[1]),
                                   [0, BLK]])
                d_q = sb.tile([64, CHUNK], F32, name=f"d{q}")
                nc.vector.tensor_sub(
                    d_q.rearrange("p (b j) -> p b j", j=BLK),
                    pa.rearrange("p (b j) -> p b j", j=BLK),
                    rrep)
                band_q = sb.tile([64, CHUNK], F32, name=f"bq{q}")
                nc.scalar.activation(band_q, d_q,
                                     mybir.ActivationFunctionType.Exp)
                nc.gpsimd.dma_start(out=bands[:, qs], in_=band_q)

    nc.compile()
    return nc


def _get_nc():
    global _NC_CACHE
    if _NC_CACHE is None:
        _NC_CACHE = build_nc()
    return _NC_CACHE


def pack_inputs(xt_core, W1, b1, W2, b2, W3, b3):
    import ml_dtypes
    bf = ml_dtypes.bfloat16
    xT = xt_core.T.astype(np.float32)          # [2, 1024]
    x_hi = xT.astype(bf)
    x_lo = (xT - x_hi.astype(np.float32)).astype(bf)
    W1_hi = W1.astype(bf)
    W1_lo = (W1 - W1_hi.astype(np.float32)).astype(bf)
    b1_hi = b1.astype(bf)
    b1_lo = (b1 - b1_hi.astype(np.float32)).astype(bf)
    xws = np.zeros((10, XWS_COLS), dtype=bf)
    xws[0:2, :AGENTS] = x_hi
    xws[2:4, :AGENTS] = x_lo
    xws[4:6, :AGENTS] = x_hi
    xws[6:8, :AGENTS] = x_lo
    xws[8:10, :AGENTS] = np.ones((2, AGENTS), dtype=bf)
    xws[0:2, AGENTS:] = W1_hi
    xws[2:4, AGENTS:] = W1_hi
    xws[4:6, AGENTS:] = W1_lo
    xws[6:8, AGENTS:] = W1_lo
    xws[8, AGENTS:] = b1_hi
    xws[9, AGENTS:] = b1_lo
    wb = np.zeros((65, WB_COLS), dtype=np.float32)
    wb[0:64, 0:64] = W3
    wb[64, 0:64] = b3
    wb[0:32, 64:128] = W2
    wb[32, 64:128] = b2
    return xws, wb


def kernel(x, W1, b1, W2, b2, W3, b3, sub_batches, **run_kwargs):
    global LAST_RESULT
    x = np.asarray(x)
    xt = np.ascontiguousarray(x[:, -1, :], dtype=np.float32)  # [8192, 2]
    W1 = np.asarray(W1, dtype=np.float32)
    W2 = np.asarray(W2, dtype=np.float32)
    W3 = np.asarray(W3, dtype=np.float32)
    b1 = np.asarray(b1, dtype=np.float32)
    b2 = np.asarray(b2, dtype=np.float32)
    b3 = np.asarray(b3, dtype=np.float32)

    ones = np.ones((1, AGENTS), dtype=np.float32)
    in_maps = []
    for d in range(NCORES):
        xws, wb = pack_inputs(
            xt[d * AGENTS:(d + 1) * AGENTS, :], W1, b1, W2, b2, W3, b3)
        in_maps.append({"xws": xws, "wb": wb, "ones": ones})

    nc = _get_nc()
    res = run_bass_kernel_spmd(nc, in_maps, core_ids=list(range(NCORES)),
                               **run_kwargs)
    LAST_RESULT = res

    # Device ships G = exp(A - rowmax); the reference E = exp(A - m[j]) is
    # G^T per block (A symmetric). Paste each block transposed and apply the
    # row normalization E / (sum + eps) while assembling the zero canvas.
    full = np.zeros((BS, BS), dtype=np.float32)
    for d in range(NCORES):
        bd = np.asarray(res.results[d]["bands"])        # [64, 1024] = G
        for b in range(16):
            n = d * 16 + b                              # global 64-row block
            E = np.ascontiguousarray(bd[:, b * BLK:(b + 1) * BLK].T)
            P = E / (E.sum(axis=1, keepdims=True) + EPS)
            full[n * BLK:(n + 1) * BLK, n * BLK:(n + 1) * BLK] = P

    starts = np.asarray(sub_batches)[:, 0]
    canonical = np.array_equal(starts, np.arange(128, dtype=np.int64) * BLK)
    if not canonical:
        # General placement: extract the 64x64 blocks and scatter them at the
        # rows given by sub_batches (faithful to the reference .at[].set).
        scat = np.zeros((BS, BS), dtype=np.float32)
        for n in range(128):
            blk = full[n * BLK:(n + 1) * BLK, n * BLK:(n + 1) * BLK]
            rows = int(starts[n]) + np.arange(BLK)
            scat[np.ix_(rows, rows)] = blk
        full = scat
    return full
